# revision 1
# baseline (speedup 1.0000x reference)
"""Trainium2 Bass kernel for the ConvBranch (Mamba-style) model.

Sharding: 8 cores = 4 batches x 2 DI-halves.
  core c -> batch b = c//2, half m = c%2 (owns DI channels [m*512,(m+1)*512)).
Dense matmuls (in_proj/conv/x_proj/out_proj) are replicated within a pair;
the selective-scan trio (dA, dBu, scan, y*C) is sharded by DI-half; gated y
halves are exchanged with one AllGather per layer.

Per-core channel order is permuted so the core's OWN half occupies xi tiles
0..3 (keeps the SPMD program identical across cores; host permutes weights
to match).

Layouts on chip: feature on partitions, time on free dim.
Scan tiles: partition p = 16*j + n (j = d-sub 0..7, n = state 0..15),
64 scan tiles per layer each [128, 512].
"""

import sys

sys.path.insert(0, "/opt/trn_rl_repo")

from contextlib import ExitStack

import numpy as np
import ml_dtypes

import concourse.bass as bass
import concourse.bacc as bacc
import concourse.tile as tile
from concourse import mybir
from concourse.bass_utils import run_bass_kernel_spmd

F32 = mybir.dt.float32
BF16 = mybir.dt.bfloat16
NPBF16 = ml_dtypes.bfloat16
AF = mybir.ActivationFunctionType
OP = mybir.AluOpType

B, T, F = 4, 2048, 128
DM, L, STRIDE, KF = 512, 4, 4, 2
N, DC, E = 16, 4, 2
DI = E * DM            # 1024
R = (DM + 15) // 16    # 32
K = KF * STRIDE        # 8
TS = T // STRIDE       # 512
EPS = 1e-5
DH = DI // 2           # 512 channels per core half
NC_CORES = 8
GROUPS = [[0, 1], [2, 3], [4, 5], [6, 7]]

_CACHE = {}
_DEBUG = False
_SIM_FUNCS = False  # swap Gelu/Silu for sim-supported funcs (timing runs)


def _emit(ctx, tc, ins, out, dbgs=None):
    nc = tc.nc
    af_gelu = AF.Tanh if _SIM_FUNCS else AF.Gelu
    af_silu = AF.Sigmoid if _SIM_FUNCS else AF.Silu

    def dbg(name, ap):
        if dbgs is None:
            return
        t = nc.dram_tensor("dbg_" + name, list(ap.shape), ap.dtype,
                           kind="ExternalOutput")
        nc.sync.dma_start(out=t[...], in_=ap)
        dbgs.append("dbg_" + name)

    consts = ctx.enter_context(tc.tile_pool(name="consts", bufs=1))
    wpool = ctx.enter_context(tc.tile_pool(name="wpool", bufs=2))
    work = ctx.enter_context(tc.tile_pool(name="work", bufs=1))
    scanp = ctx.enter_context(tc.tile_pool(name="scanp", bufs=3))
    ppool = ctx.enter_context(tc.tile_pool(name="ppool", bufs=2, space="PSUM"))
    pw = ctx.enter_context(tc.tile_pool(name="pw", bufs=2, space="PSUM"))
    py = ctx.enter_context(tc.tile_pool(name="py", bufs=2, space="PSUM"))
    dram = ctx.enter_context(tc.tile_pool(name="dram", bufs=2, space="DRAM"))

    # ---- persistent constants ----
    xt_sb = consts.tile([128, T + K - 1], BF16)
    nc.gpsimd.dma_start(out=xt_sb, in_=ins["xt"][:, :])
    w1t_sb = consts.tile([128, K, DM], BF16)
    nc.gpsimd.dma_start(out=w1t_sb, in_=ins["w1t"].rearrange("k f m -> f k m"))
    cb_sb = consts.tile([128, 4], F32)
    nc.gpsimd.dma_start(out=cb_sb, in_=ins["cb"][:, :])
    ident_sb = consts.tile([128, 128], F32)
    nc.gpsimd.dma_start(out=ident_sb, in_=ins["ident"][:, :])
    indrep_sb = consts.tile([128, 16, 128], BF16)
    nc.gpsimd.dma_start(out=indrep_sb, in_=ins["indrep"].rearrange("g p m -> p g m"))
    indred_sb = consts.tile([128, 16, 128], BF16)
    nc.gpsimd.dma_start(out=indred_sb, in_=ins["indred"].rearrange("g p m -> p g m"))
    indbc_sb = consts.tile([64, 2, 128], BF16)
    nc.gpsimd.dma_start(out=indbc_sb, in_=ins["indbc"].rearrange("c p m -> p c m"))
    ones128_sb = consts.tile([128, 1], BF16)
    nc.vector.memset(ones128_sb, 1.0)
    ones1_sb = consts.tile([1, 128], BF16)
    nc.vector.memset(ones1_sb, 1.0)
    eps1_sb = consts.tile([1, 1], F32)
    nc.vector.memset(eps1_sb, EPS)
    onesf_sb = consts.tile([128, 1], F32)
    nc.vector.memset(onesf_sb, 1.0)
    nw_sb = consts.tile([128, 4], F32)
    nc.gpsimd.dma_start(out=nw_sb, in_=ins["nw"][:, :])
    nb_sb = consts.tile([128, 4], F32)
    nc.gpsimd.dma_start(out=nb_sb, in_=ins["nb"][:, :])

    # residual stream h: 4 persistent fp32 tiles [128, TS]
    h = [consts.tile([128, TS], F32, name=f"h{kt}") for kt in range(4)]

    # ---- front conv + gelu ----
    for mt in range(4):
        ps = ppool.tile([128, TS], F32, name="ps")
        for k in range(K):
            nc.tensor.matmul(
                ps,
                w1t_sb[:, k, mt * 128:(mt + 1) * 128],
                xt_sb[:, k:k + T:STRIDE],
                start=(k == 0),
                stop=(k == K - 1),
            )
        nc.scalar.activation(h[mt], ps, af_gelu, bias=cb_sb[:, mt:mt + 1])
        dbg(f"hfront{mt}", h[mt])

    def layernorm(lw_ap, lb_ap, out_dtype, name):
        """LN over the partition (feature) dim of h; returns 4 tiles [128,TS]."""
        hb = []
        st_m = ppool.tile([128, TS], F32, name="ps")
        st_q = ppool.tile([128, TS], F32, name="ps")
        for kt in range(4):
            hbt = work.tile([128, TS], BF16, name=f"hb{kt}")
            nc.scalar.copy(hbt, h[kt])
            hb.append(hbt)
            sq = work.tile([128, TS], BF16, name="sq")
            nc.scalar.activation(sq, h[kt], AF.Square)
            nc.tensor.matmul(st_m[0:1, :], ones128_sb, hbt,
                             start=(kt == 0), stop=(kt == 3))
            nc.tensor.matmul(st_q[0:1, :], ones128_sb, sq,
                             start=(kt == 0), stop=(kt == 3))
        ms = work.tile([1, 2 * TS], F32, name="ms")
        nc.vector.tensor_scalar(ms[:, 0:TS], st_m[0:1, :], 1.0 / DM, None, OP.mult)
        nc.vector.tensor_scalar(ms[:, TS:2 * TS], st_q[0:1, :], 1.0 / DM, None,
                                OP.mult)
        mu2 = work.tile([1, TS], F32, name="mu2")
        nc.vector.tensor_tensor(out=mu2, in0=ms[:, 0:TS], in1=ms[:, 0:TS],
                                op=OP.mult)
        var = work.tile([1, TS], F32, name="var")
        nc.vector.tensor_tensor(out=var, in0=ms[:, TS:2 * TS], in1=mu2,
                                op=OP.subtract)
        # rstd = exp(-0.5 * ln(var + eps)); c = mu * rstd   (packed [1, 2*TS])
        rc = work.tile([1, 2 * TS], F32, name="rc")
        lnv = work.tile([1, TS], F32, name="lnv")
        nc.scalar.activation(lnv, var, AF.Ln, bias=eps1_sb[:, 0:1])
        nc.scalar.activation(rc[:, 0:TS], lnv, AF.Exp, scale=-0.5)
        nc.vector.tensor_tensor(out=rc[:, TS:2 * TS], in0=ms[:, 0:TS],
                                in1=rc[:, 0:TS], op=OP.mult)
        rcb = work.tile([1, 2 * TS], BF16, name="rcb")
        nc.vector.tensor_copy(out=rcb, in_=rc)
        psr = ppool.tile([128, TS], F32, name="ps")
        nc.tensor.matmul(psr, ones1_sb, rcb[:, 0:TS], start=True, stop=True)
        rstd_b = work.tile([128, TS], F32, name="rstd_b")
        nc.scalar.copy(rstd_b, psr)
        psc2 = ppool.tile([128, TS], F32, name="ps")
        nc.tensor.matmul(psc2, ones1_sb, rcb[:, TS:2 * TS], start=True, stop=True)
        c_b = work.tile([128, TS], F32, name="c_b")
        nc.scalar.copy(c_b, psc2)
        outs = []
        for kt in range(4):
            t1 = work.tile([128, TS], F32, name="lnt1")
            nc.vector.tensor_tensor(out=t1, in0=h[kt], in1=rstd_b, op=OP.mult)
            t2 = work.tile([128, TS], F32, name="lnt2")
            nc.vector.tensor_tensor(out=t2, in0=t1, in1=c_b, op=OP.subtract)
            o = work.tile([128, TS], out_dtype, name=f"{name}{kt}")
            nc.vector.tensor_scalar(o, t2, lw_ap[:, kt:kt + 1], lb_ap[:, kt:kt + 1],
                                    OP.mult, OP.add)
            outs.append(o)
        return outs

    for l in range(L):
        # ---- per-layer weights ----
        wint_sb = wpool.tile([128, 4, DI + DH], BF16, name="wint")
        nc.gpsimd.dma_start(out=wint_sb,
                          in_=ins["wint"][l].rearrange("(kt p) e -> p kt e", p=128))
        wot_sb = wpool.tile([128, 8, DM], BF16, name="wot")
        nc.gpsimd.dma_start(out=wot_sb,
                          in_=ins["wot"][l].rearrange("(kd p) o -> p kd o", p=128))
        xpt_sb = wpool.tile([128, 8, R + 2 * N], BF16, name="xpt")
        nc.gpsimd.dma_start(out=xpt_sb,
                          in_=ins["xpt"][l].rearrange("(kd p) e -> p kd e", p=128))
        dtpt_sb = wpool.tile([32, DH], BF16, name="dtpt")
        nc.gpsimd.dma_start(out=dtpt_sb, in_=ins["dtpt"][l])
        cw_sb = wpool.tile([128, 8, DC], F32, name="cw")
        nc.gpsimd.dma_start(out=cw_sb,
                          in_=ins["cw1d"][l].rearrange("(et p) j -> p et j", p=128))
        cb1_sb = wpool.tile([128, 8], F32, name="cb1")
        nc.gpsimd.dma_start(out=cb1_sb, in_=ins["cb1d"][l])
        dtpb_sb = wpool.tile([128, 4], F32, name="dtpb")
        nc.gpsimd.dma_start(out=dtpb_sb, in_=ins["dtpb"][l])
        asc_sb = wpool.tile([128, 64], F32, name="asc")
        nc.gpsimd.dma_start(out=asc_sb, in_=ins["asc"][l])
        dsc_sb = wpool.tile([128, 4], F32, name="dsc")
        nc.gpsimd.dma_start(out=dsc_sb, in_=ins["dsc"][l])
        lnw_sb = wpool.tile([128, 4], F32, name="lnw")
        nc.gpsimd.dma_start(out=lnw_sb, in_=ins["lnw"][l])
        lnb_sb = wpool.tile([128, 4], F32, name="lnb")
        nc.gpsimd.dma_start(out=lnb_sb, in_=ins["lnb"][l])

        # ---- LN ----
        hn = layernorm(lnw_sb, lnb_sb, BF16, "hn")
        if l == 0:
            dbg("hn0", hn[0])

        # ---- in_proj: xi tiles 0..7 (padded for conv), z tiles 8..11 ----
        xi_pad = []
        zs = []
        for et in range(12):
            ps = ppool.tile([128, TS], F32, name="ps")
            for kt in range(4):
                nc.tensor.matmul(ps, wint_sb[:, kt, et * 128:(et + 1) * 128], hn[kt],
                                 start=(kt == 0), stop=(kt == 3))
            if et < 8:
                xp = work.tile([128, DC - 1 + TS], BF16, name=f"xipad{et}")
                nc.vector.memset(xp[:, 0:DC - 1], 0.0)
                nc.scalar.copy(xp[:, DC - 1:DC - 1 + TS], ps)
                if l == 0 and et == 0:
                    dbg("xipad0", xp)
                xi_pad.append(xp)
            else:
                z = work.tile([128, TS], BF16, name=f"zs{et - 8}")
                nc.scalar.activation(z, ps, af_silu)
                if l == 0 and et == 8:
                    dbg("zs0", z)
                zs.append(z)

        # ---- causal depthwise conv1d + silu ----
        xi = []
        for et in range(8):
            acc0 = work.tile([128, TS], BF16, name="cacc0")
            nc.vector.tensor_scalar(acc0, xi_pad[et][:, 0:TS],
                                    cw_sb[:, et, 0:1], None, OP.mult)
            acc1 = work.tile([128, TS], BF16, name="cacc1")
            nc.vector.scalar_tensor_tensor(acc1, xi_pad[et][:, 1:1 + TS],
                                           cw_sb[:, et, 1:2], acc0, OP.mult, OP.add)
            acc2 = work.tile([128, TS], BF16, name="cacc2")
            nc.vector.scalar_tensor_tensor(acc2, xi_pad[et][:, 2:2 + TS],
                                           cw_sb[:, et, 2:3], acc1, OP.mult, OP.add)
            acc3 = work.tile([128, TS], BF16, name="cacc3")
            nc.vector.scalar_tensor_tensor(acc3, xi_pad[et][:, 3:3 + TS],
                                           cw_sb[:, et, 3:4], acc2, OP.mult, OP.add)
            xit = work.tile([128, TS], BF16, name=f"xi{et}")
            nc.scalar.activation(xit, acc3, af_silu, bias=cb1_sb[:, et:et + 1])
            if l == 0 and et == 0:
                dbg("xi0", xit)
            xi.append(xit)

        # ---- x_proj -> x_dbl [64, TS]; rows 0:32 dt_raw, 32:48 B, 48:64 C ----
        psx = ppool.tile([64, TS], F32, name="ps")
        for et in range(8):
            nc.tensor.matmul(psx, xpt_sb[:, et, :], xi[et],
                             start=(et == 0), stop=(et == 7))
        xdbl = work.tile([64, TS], BF16, name="xdbl")
        nc.scalar.copy(xdbl, psx)
        if l == 0:
            dbg("xdbl", xdbl)

        # broadcast B, C across partition groups: row 16*j + n <- B[n] / C[n]
        # via PE indicator matmuls (DMA repeat-dims explode into many SP ops)
        psb = ppool.tile([128, TS], F32, name="ps")
        nc.tensor.matmul(psb, indbc_sb[:, 0, :], xdbl,
                         start=True, stop=True)
        brep = work.tile([128, TS], BF16, name="brep")
        nc.scalar.copy(brep, psb)
        psc = ppool.tile([128, TS], F32, name="ps")
        nc.tensor.matmul(psc, indbc_sb[:, 1, :], xdbl,
                         start=True, stop=True)
        crep = work.tile([128, TS], BF16, name="crep")
        nc.scalar.copy(crep, psc)
        if l == 0:
            dbg("brep", brep)
            dbg("crep", crep)

        # ---- dt: softplus(dt_proj @ dt_raw + b) = ln(1 + exp(.)) ----
        w = []
        dtu = []
        for kt in range(4):
            psd = ppool.tile([128, TS], F32, name="ps")
            nc.tensor.matmul(psd, dtpt_sb[:, kt * 128:(kt + 1) * 128],
                             xdbl[0:32, :], start=True, stop=True)
            edt = work.tile([128, TS], F32, name="edt")
            nc.scalar.activation(edt, psd, AF.Exp, bias=dtpb_sb[:, kt:kt + 1])
            if l == 0 and kt == 0:
                dbg("edt0", edt)
            wt = work.tile([128, TS], BF16, name=f"w{kt}")
            nc.scalar.activation(wt, edt, AF.Ln, bias=onesf_sb[:, 0:1])
            if l == 0 and kt == 0:
                dbg("w0", wt)
            w.append(wt)
            du = work.tile([128, TS], BF16, name=f"dtu{kt}")
            nc.vector.tensor_tensor(out=du, in0=wt, in1=xi[kt], op=OP.mult)
            if l == 0 and kt == 0:
                dbg("dtu0", du)
            dtu.append(du)

        # ---- scan + gating per d-tile ----
        yg = work.tile([128, 4, TS], BF16, name="yg")
        for kt in range(4):
            pyt = py.tile([128, TS], F32, name="py")
            for g in range(16):
                # w_rep via PE indicator matmul -> psum
                pwr = pw.tile([128, TS], F32, name="pw")
                nc.tensor.matmul(pwr, indrep_sb[:, g, :], w[kt],
                                 start=True, stop=True)
                # dA = exp(A * w_rep) straight from psum
                dA = scanp.tile([128, TS], BF16, name="dA")
                s = kt * 16 + g
                nc.scalar.activation(dA, pwr, AF.Exp, scale=asc_sb[:, s:s + 1])
                # dtu_rep via PE indicator matmul -> psum
                pr = pw.tile([128, TS], F32, name="pr")
                nc.tensor.matmul(pr, indrep_sb[:, g, :], dtu[kt],
                                 start=True, stop=True)
                dBu = scanp.tile([128, TS], BF16, name="dBu")
                nc.vector.tensor_tensor(out=dBu, in0=pr, in1=brep, op=OP.mult)
                hs = scanp.tile([128, TS], BF16, name="hs")
                nc.vector.tensor_tensor_scan(hs, dA, dBu, 0.0, OP.mult, OP.add)
                if l == 0 and kt == 0 and g == 0:
                    dbg("dA00", dA)
                    dbg("dBu00", dBu)
                    dbg("hs00", hs)
                tmp = scanp.tile([128, TS], BF16, name="tmp")
                nc.gpsimd.tensor_tensor(out=tmp, in0=hs, in1=crep, op=OP.mult)
                if l == 0 and kt == 0 and g == 0:
                    dbg("tmp00", tmp)
                nc.tensor.matmul(pyt, indred_sb[:, g, :], tmp,
                                 start=(g == 0), stop=(g == 15))
            # gating: yg = (y + xi * D) * silu(z)
            g1 = work.tile([128, TS], F32, name="g1")
            nc.vector.scalar_tensor_tensor(g1, xi[kt], dsc_sb[:, kt:kt + 1],
                                           pyt, OP.mult, OP.add)
            nc.gpsimd.tensor_tensor(out=yg[:, kt, :], in0=g1, in1=zs[kt],
                                    op=OP.mult)
            if l == 0 and kt == 0:
                dbg("g10", g1)

        # ---- exchange gated y halves ----
        ccin = dram.tile([DH, TS], BF16, name="ccin")
        nc.sync.dma_start(out=ccin.rearrange("(kt p) t -> p kt t", p=128), in_=yg)
        ccout = dram.tile([DI, TS], BF16, name="ccout")
        nc.gpsimd.collective_compute(
            "AllGather", OP.bypass, replica_groups=GROUPS,
            ins=[ccin[:, :]], outs=[ccout[:, :]],
        )
        ygf = work.tile([128, 8, TS], BF16, name="ygf")
        nc.sync.dma_start(out=ygf,
                          in_=ccout.rearrange("(kd p) t -> p kd t", p=128))
        if l == 0:
            dbg("ygf", ygf)

        # ---- out_proj + residual ----
        for mt in range(4):
            pso = ppool.tile([128, TS], F32, name="ps")
            for kd in range(8):
                nc.tensor.matmul(pso, wot_sb[:, kd, mt * 128:(mt + 1) * 128],
                                 ygf[:, kd, :], start=(kd == 0), stop=(kd == 7))
            nc.vector.tensor_tensor(out=h[mt], in0=h[mt], in1=pso, op=OP.add)
            if l == 0 and mt == 0:
                dbg("hl0", h[0])

    # ---- final LN ----
    hnf = layernorm(nw_sb, nb_sb, F32, "hnf")

    # ---- transpose + repeat-interleave upsample + store ----
    for ct in range(4):
        hT = work.tile([128, DM], F32, name=f"hT{ct}")
        for kt in range(4):
            pt = ppool.tile([128, 128], F32, name="ps")
            nc.tensor.transpose(pt, hnf[kt][:, ct * 128:(ct + 1) * 128], ident_sb)
            nc.vector.tensor_copy(out=hT[:, kt * 128:(kt + 1) * 128], in_=pt)
        for j in range(STRIDE):
            base = 512 * ct + j
            nc.sync.dma_start(out=out[base:base + 509:STRIDE, :], in_=hT)


def _build_nc():
    nc = bacc.Bacc("TRN2", num_devices=NC_CORES)
    ins = {}

    def din(name, shape, dt):
        ins[name] = nc.dram_tensor(name, list(shape), dt, kind="ExternalInput")

    din("xt", (128, T + K - 1), BF16)
    din("w1t", (K, 128, DM), BF16)
    din("cb", (128, 4), F32)
    din("ident", (128, 128), F32)
    din("indrep", (16, 128, 128), BF16)
    din("indred", (16, 128, 128), BF16)
    din("indbc", (2, 64, 128), BF16)
    din("nw", (128, 4), F32)
    din("nb", (128, 4), F32)
    din("wint", (L, DM, DI + DH), BF16)
    din("wot", (L, DI, DM), BF16)
    din("xpt", (L, DI, R + 2 * N), BF16)
    din("dtpt", (L, R, DH), BF16)
    din("cw1d", (L, DI, DC), F32)
    din("cb1d", (L, 128, 8), F32)
    din("dtpb", (L, 128, 4), F32)
    din("asc", (L, 128, 64), F32)
    din("dsc", (L, 128, 4), F32)
    din("lnw", (L, 128, 4), F32)
    din("lnb", (L, 128, 4), F32)
    out = nc.dram_tensor("out", [T, DM], F32, kind="ExternalOutput")

    dbgs = [] if _DEBUG else None
    with ExitStack() as ctx:
        tc = ctx.enter_context(tile.TileContext(nc))
        _emit(ctx, tc, ins, out, dbgs)
    nc.compile()
    _CACHE["dbgs"] = dbgs
    return nc


def _prep_core_inputs(c, inputs):
    b, m = c // 2, c % 2
    bf = lambda a: np.ascontiguousarray(a).astype(NPBF16)
    f32 = lambda a: np.ascontiguousarray(a).astype(np.float32)

    x = np.asarray(inputs["x"], np.float32)
    xt = np.zeros((128, T + K - 1), np.float32)
    xt[:, K - 1:] = x[b].T
    w1t = np.asarray(inputs["conv_w"], np.float32).transpose(2, 1, 0)  # [K,F,DM]
    cb = np.asarray(inputs["conv_b"], np.float32).reshape(4, 128).T
    ident = np.eye(128, dtype=np.float32)
    indrep = np.zeros((16, 128, 128), np.float32)
    indred = np.zeros((16, 128, 128), np.float32)
    for g in range(16):
        for j in range(8):
            for n in range(16):
                indrep[g, 8 * g + j, 16 * j + n] = 1.0
                indred[g, 16 * j + n, 8 * g + j] = 1.0
    indbc = np.zeros((2, 64, 128), np.float32)
    for j in range(8):
        for n in range(16):
            indbc[0, 32 + n, 16 * j + n] = 1.0
            indbc[1, 48 + n, 16 * j + n] = 1.0
    nw = np.asarray(inputs["norm_w"], np.float32).reshape(4, 128).T
    nb = np.asarray(inputs["norm_b"], np.float32).reshape(4, 128).T

    # per-core DI channel permutation: own half first
    own = np.arange(m * DH, (m + 1) * DH)
    oth = np.arange((1 - m) * DH, (2 - m) * DH)
    perm = np.concatenate([own, oth])

    in_w = np.asarray(inputs["in_proj_w"], np.float32)    # [L, 2*DI, DM]
    wint = np.empty((L, DM, DI + DH), np.float32)
    for l in range(L):
        wt = in_w[l].T                                    # [DM, 2*DI]
        wint[l, :, :DI] = wt[:, perm]                     # xi, permuted
        wint[l, :, DI:] = wt[:, DI + own]                 # z own half
    wot = np.asarray(inputs["out_proj_w"], np.float32).transpose(0, 2, 1)  # [L,DI,DM]
    xpt = np.asarray(inputs["x_proj_w"], np.float32).transpose(0, 2, 1)[:, perm, :]
    dtpt = np.asarray(inputs["dt_proj_w"], np.float32).transpose(0, 2, 1)[:, :, own]
    cw1d = np.asarray(inputs["conv1d_w"], np.float32)[:, perm, :]
    cb1d = np.asarray(inputs["conv1d_b"], np.float32)[:, perm].reshape(L, 8, 128)
    cb1d = cb1d.transpose(0, 2, 1)
    dtpb = np.asarray(inputs["dt_proj_b"], np.float32)[:, own].reshape(L, 4, 128)
    dtpb = dtpb.transpose(0, 2, 1)
    A = -np.exp(np.asarray(inputs["A_log"], np.float32))[:, own, :]  # [L, DH, N]
    asc = A.reshape(L, 4, 16, 8, 16).transpose(0, 3, 4, 1, 2).reshape(L, 128, 64)
    dsc = np.asarray(inputs["D_skip"], np.float32)[:, own].reshape(L, 4, 128)
    dsc = dsc.transpose(0, 2, 1)
    lnw = np.asarray(inputs["ln_w"], np.float32).reshape(L, 4, 128).transpose(0, 2, 1)
    lnb = np.asarray(inputs["ln_b"], np.float32).reshape(L, 4, 128).transpose(0, 2, 1)

    return dict(
        xt=bf(xt), w1t=bf(w1t), cb=f32(cb), ident=ident,
        indrep=bf(indrep), indred=bf(indred), indbc=bf(indbc),
        nw=f32(nw), nb=f32(nb),
        wint=bf(wint), wot=bf(wot), xpt=bf(xpt), dtpt=bf(dtpt),
        cw1d=f32(cw1d), cb1d=f32(cb1d), dtpb=f32(dtpb), asc=f32(asc),
        dsc=f32(dsc), lnw=f32(lnw), lnb=f32(lnb),
    )


def kernel(trace=False, **inputs):
    if "nc" not in _CACHE:
        _CACHE["nc"] = _build_nc()
    nc = _CACHE["nc"]
    in_maps = [_prep_core_inputs(c, inputs) for c in range(NC_CORES)]
    res = run_bass_kernel_spmd(nc, in_maps, list(range(NC_CORES)), trace=trace)
    out = np.stack([np.asarray(res.results[2 * b]["out"], np.float32)
                    for b in range(B)])
    _CACHE["last_result"] = res
    return out

